# revision 60
# baseline (speedup 1.0000x reference)
"""MoE MLP (Mixtral-style top-2 routing) on 8 Trainium2 NeuronCores.

Strategy: expert-COLUMN-parallel (F-sharding) with fp8 split-compensation
matmuls. The router (tiny: T x H x E) runs on host in fp32, exactly
mirroring the reference math. Tokens are grouped by expert on host; EVERY
core processes EVERY expert's token group, but only a 512-wide slice of
the F dimension (core k owns F rows [k*512,(k+1)*512) of each expert's
Wg/Wu and the matching Wd columns). Per-core work is identical by
construction. Each core produces a PARTIAL down projection; the host sums
the 8 partials and applies the top-k combine weights in a weighted
scatter-add.

fp8 split-compensation: every operand X (weights on host, h on host, the
activation a on device) is represented as X = X1 + X2/S with X1 = fp8(X),
X2 = fp8(S*(X - X1)), S = 32 (16 for the a-residual; power-of-two scales
are exact in fp8). A logical 128-contraction matmul A^T B then needs the
three products A1B1 + A1B2 + A2B1 (the lo*lo term is ~0.13% of a percent
and dropped). DoubleRow fp8 matmuls compute TWO independent 128-
contraction products per instruction at 0.5 cycles/column, so per k-tile
the split costs 1.5 DR slots = 0.75x the f32r time:
  main DR (per k-PAIR):  (S*A1[k], S*A1[k+1]) x (B1[k], B1[k+1])
  cross DR (per k-tile): (A1[k], S*A2[k])     x (S*B2[k], B1[k])
All terms land uniformly scaled by S in the PSUM group. Scale bookkeeping:
  gate psum = 32*g   (silu reads it with ACT scale=1/32)
  up weights pre-scaled by 1/4 on host -> up psum = 8*u
  a_bar = silu(g) * u_psum = 8*a  (DVE mul, scale comes for free)
  a1b = fp8(a_bar) = 8*a1;  a2b = fp8(a_bar - a1b) = 8*(a-a1)  [DVE sub]
  a1  = fp8(a_bar * 1/8)  [ACT copy, scale=0.125]
  down weights pre-scaled by 4 on host -> down psum = 32*y
  y_sb = psum * 1/32 (DVE tensor_scalar_mul), stored bf16
End-to-end quantization error ~0.3-0.4% (budget 2e-2).

Program: one weight BLOCK per expert (double-buffered); each expert's
tokens form one pass (<=2304 resident tokens, next pass's first h tile
prefetched via h_pre); per pass a loop over near-equal <=512-wide ct
tiles. The PE stream runs one ct-tile ahead of the silu/mul/split stage;
the DVE stream orders [muls(c+1), y-copies(c), subs(c+1)] so the psum
bank copies start the moment down(c) begins; y-copies go out singles-
then-pairs to stay ahead of the PE's 0.64us/ht down cadence.
"""

import numpy as np
import ml_dtypes
import concourse.bass as bass
import concourse.mybir as mybir
from concourse.bass_utils import run_bass_kernel_spmd

f32 = mybir.dt.float32
fp8 = mybir.dt.float8e4
bf16 = mybir.dt.bfloat16
E4M3 = ml_dtypes.float8_e4m3fn
ACT_FUNC = mybir.ActivationFunctionType.Silu  # swapped in coresim_check.py

B, S, H, F, E = 4, 2048, 1024, 4096, 8
KT = H // 128  # 8 k-tiles of the H contraction
KP = KT // 2  # 4 k-pairs for the main-term DRs
FSH = F // 8  # 512-wide per-core F slice
FT_PER = FSH // 128  # 4 f-tiles per slice
FPAIR = FT_PER // 2  # 2 f-pairs for the down main-term DRs
HT = H // 128  # 8 output H tiles
CT_W = 512  # max token tile width (moving dim N)
PASS_MAX = 2304  # SBUF budget for h_sb/y_sb columns (one pass per expert)
SC = 32.0  # residual scale (power of two)
DR = mybir.MatmulPerfMode.DoubleRow
WARM_N1 = 13  # dummy matmuls spanning the pre-first-weight DMA wait
WARM_PAD = 0  # dummies padding ctg0 feed stalls (0 = disabled)


def _split_tiles(pass_size, lead256=False):
    """Split a pass into near-equal ct tiles (<=512 wide, multiples of 32):
    equal widths avoid narrow trailing tiles whose short gu window can't
    hide the act-chain latency. lead256: a 256 first tile (pass 0 only)
    shrinks the h DMA on the startup critical path."""
    widths = []
    rest = pass_size
    if lead256 and rest >= 1024:
        widths.append(256)
        rest -= 256
    n = -(-rest // 512)
    base = (rest // n) // 32 * 32
    ws = [base] * n
    rem = (rest - base * n) // 32
    for i in range(rem):
        ws[i] += 32
    widths += ws
    assert sum(widths) == pass_size and all(256 <= w <= 512 for w in widths), widths
    return widths


def _expert_passes(ce_pad):
    """Split one expert's padded token count into passes of <=PASS_MAX,
    near-equal, multiples of 32."""
    n = -(-ce_pad // PASS_MAX)
    base = (ce_pad // n) // 32 * 32
    out = [base] * n
    rem = (ce_pad - base * n) // 32
    for i in range(rem):
        out[i] += 32
    assert sum(out) == ce_pad and all(256 <= ps <= PASS_MAX for ps in out), out
    return out


def build_program(expert_sizes, repeat=1):
    """Per-core Bass program. `expert_sizes`: padded token count per
    present expert (in block order). Each expert is one weight block
    spanning 1+ token passes. `repeat` re-runs everything (bench only)."""
    NWB = len(expert_sizes)
    pass_sizes = []
    pass_wb = []  # weight block (expert slot) per pass
    for b, ce in enumerate(expert_sizes):
        for ps in _expert_passes(ce):
            pass_sizes.append(ps)
            pass_wb.append(b)
    TC = sum(pass_sizes)
    tok0 = [sum(pass_sizes[:p]) for p in range(len(pass_sizes))]

    pass_sizes = pass_sizes * repeat
    pass_tok0 = tok0 * repeat
    pass_wb = [b + r * NWB for r in range(repeat) for b in pass_wb]
    NP = len(pass_sizes)
    NB = NWB * repeat  # global weight-block sequence length
    PSMAX = max(pass_sizes)
    tiles = [_split_tiles(ps) for ps in pass_sizes]
    NCT = [len(t) for t in tiles]
    tile_offs = [[sum(tiles[p][:i]) for i in range(NCT[p])] for p in range(NP)]

    # ctg enumeration: for p, for ct -> (p, ct, width, offset)
    ctg_base = [0] * (NP + 1)
    for p in range(NP):
        ctg_base[p + 1] = ctg_base[p] + NCT[p]
    TOTAL_CT = ctg_base[NP]
    ctg_pfc = []
    for p in range(NP):
        for ct in range(NCT[p]):
            ctg_pfc.append((p, ct, tiles[p][ct], tile_offs[p][ct]))

    # last ctg (exclusive) of each weight block
    blk_pass_last = {}
    for p in range(NP):
        blk_pass_last[pass_wb[p]] = p
    blk_ctg_end = {b: ctg_base[blk_pass_last[b] + 1] for b in blk_pass_last}
    blk_pass_first = {}
    for p in range(NP - 1, -1, -1):
        blk_pass_first[pass_wb[p]] = p

    SLOTS = max(NCT)

    # yupd op layout per ctg: hts [0],[1],[2],[3],[4,5],[6,7] — singles
    # while the PE's down still races ahead on fresh banks (each single
    # frees its bank ~0.8us before the down's reuse), then pairs for
    # throughput (1.26us per 2 ht < the PE's 1.28us cadence).
    YUPD_HT = [(0, 1), (1, 2), (2, 3), (3, 4), (4, 6), (6, 8)]
    # op index (1-based, within a ctg) that frees each ht's psum bank
    YUPD_FREE = {0: 1, 1: 2, 2: 3, 3: 4, 4: 5, 5: 5, 6: 6, 7: 6}
    YUPD_N = len(YUPD_HT)

    # Per-tile-SLOT h DMA counts: slot j of pass p has been loaded
    # ht_cnt[p][j] times through pass p (each slot has its own semaphore).
    ht_cnt = []
    cnt = [0] * SLOTS
    for p in range(NP):
        for j in range(NCT[p]):
            cnt[j] += 1
        ht_cnt.append(list(cnt))

    # y store counts per slot, same scheme
    yd_cnt = []
    cnt = [0] * SLOTS
    for p in range(NP):
        for j in range(NCT[p]):
            cnt[j] += 4 if p == NP - 1 else 1
        yd_cnt.append(list(cnt))

    def _overlaps(p, lo, hi):
        """Tile indices of pass p whose column range intersects [lo, hi)."""
        return [
            i
            for i, (o, w) in enumerate(zip(tile_offs[p], tiles[p]))
            if o < hi and o + w > lo
        ]

    # Weight-block thresholds: block 0 is piecewise on dedicated sems
    # (s_pg/s_pu per ft + s_wd0); blocks >= 1 alternate parity sems
    # s_w0/s_w1 (+96 each: 6 DMAs x 16).
    def swp_need(bs):
        assert bs >= 1
        n_parity = (bs + 1) // 2 if bs % 2 == 1 else bs // 2
        return 96 * n_parity

    nc = bass.Bass()
    # h: interleaved residual pairs, row = k*256 + v*128 + p (v0 = 32*h_lo,
    # v1 = h_hi), fp8
    hT = nc.declare_dram_parameter("hT", [KT * 2 * 128, TC], fp8, isOutput=False)
    # weights: partition-major blobs [128, blocks * piece]
    wgm = nc.declare_dram_parameter("wgm", [128, NWB * FT_PER * KP * 2 * 128], fp8, isOutput=False)
    wgc = nc.declare_dram_parameter("wgc", [128, NWB * FT_PER * KT * 2 * 128], fp8, isOutput=False)
    wum = nc.declare_dram_parameter("wum", [128, NWB * FT_PER * KP * 2 * 128], fp8, isOutput=False)
    wuc = nc.declare_dram_parameter("wuc", [128, NWB * FT_PER * KT * 2 * 128], fp8, isOutput=False)
    wdm = nc.declare_dram_parameter("wdm", [128, NWB * HT * FPAIR * 2 * 128], fp8, isOutput=False)
    wdc = nc.declare_dram_parameter("wdc", [128, NWB * HT * FT_PER * 2 * 128], fp8, isOutput=False)
    yT = nc.declare_dram_parameter("yT", [H, TC], bf16, isOutput=True)

    hT_v = hT.rearrange("(k v p) t -> p k v t", p=128, v=2)  # [128, KT, 2, TC]
    wgm_v = wgm.rearrange("p (b f k v c) -> p b f k v c", b=NWB, f=FT_PER, k=KP, v=2)
    wgc_v = wgc.rearrange("p (b f k v c) -> p b f k v c", b=NWB, f=FT_PER, k=KT, v=2)
    wum_v = wum.rearrange("p (b f k v c) -> p b f k v c", b=NWB, f=FT_PER, k=KP, v=2)
    wuc_v = wuc.rearrange("p (b f k v c) -> p b f k v c", b=NWB, f=FT_PER, k=KT, v=2)
    wdm_v = wdm.rearrange("p (b j k v c) -> p b j k v c", b=NWB, j=HT, k=FPAIR, v=2)
    wdc_v = wdc.rearrange("p (b j k v c) -> p b j k v c", b=NWB, j=HT, k=FT_PER, v=2)
    yT_v = yT.rearrange("(j p) t -> p j t", p=128)  # [128, HT, TC]

    from contextlib import ExitStack

    with ExitStack() as ctx:
        en = ctx.enter_context
        # h: [part, k, ver, col]; ver0 = 32*h_lo, ver1 = h_hi
        h_sb = en(nc.sbuf_tensor("h_sb", [128, KT, 2, PSMAX], fp8))
        h_pre = en(nc.sbuf_tensor("h_pre", [128, KT, 2, CT_W], fp8))
        y_sb = en(nc.sbuf_tensor("y_sb", [128, HT, PSMAX], bf16))
        wgm_sb = en(nc.sbuf_tensor("wgm_sb", [128, 2, FT_PER, KP, 2, 128], fp8))
        wgc_sb = en(nc.sbuf_tensor("wgc_sb", [128, 2, FT_PER, KT, 2, 128], fp8))
        wum_sb = en(nc.sbuf_tensor("wum_sb", [128, 2, FT_PER, KP, 2, 128], fp8))
        wuc_sb = en(nc.sbuf_tensor("wuc_sb", [128, 2, FT_PER, KT, 2, 128], fp8))
        wdm_sb = en(nc.sbuf_tensor("wdm_sb", [128, 2, HT, FPAIR, 2, 128], fp8))
        wdc_sb = en(nc.sbuf_tensor("wdc_sb", [128, 2, HT, FT_PER, 2, 128], fp8))
        # a_bar = 8*a staging (f32) and the fp8 act versions:
        # ver0 = a1b = 8*a1, ver1 = a2b = 8*(a-a1), ver2 = a1 (unscaled)
        s_sb = en(nc.sbuf_tensor("s_sb", [128, 2, FT_PER, CT_W], f32))
        act8 = en(nc.sbuf_tensor("act8", [128, 2, FT_PER, 3, CT_W], fp8))

        g_ps = [en(nc.psum_tensor(f"g_ps{i}", [128, CT_W], f32)) for i in range(2)]
        u_ps = [en(nc.psum_tensor(f"u_ps{i}", [128, CT_W], f32)) for i in range(2)]
        # one 4-bank tensor so the y copies can go out in ht-PAIRS
        yp_ps = en(nc.psum_tensor("yp_ps", [128, 4, CT_W], f32))

        warm_sb = en(nc.sbuf_tensor("warm_sb", [128, 2, 512], fp8))

        s_warm = en(nc.semaphore(name="s_warm"))  # warm_sb zeroed (DVE memset)
        s_yf = en(nc.semaphore(name="s_yf"))  # final-pass y stores (sync queue)
        s_h0 = en(nc.semaphore(name="s_h0"))  # pass-0 ct0 h, HI plane (v=1)
        s_h0b = en(nc.semaphore(name="s_h0b"))  # pass-0 ct0 h, LO plane (v=0)
        s_pg = [en(nc.semaphore(name=f"s_pg{i}")) for i in range(FT_PER)]  # blk0 wg main
        s_pgc = [en(nc.semaphore(name=f"s_pgc{i}")) for i in range(FT_PER)]  # blk0 wg cross
        s_pu = [en(nc.semaphore(name=f"s_pu{i}")) for i in range(FT_PER)]  # blk0 wu main
        s_puc = [en(nc.semaphore(name=f"s_puc{i}")) for i in range(FT_PER)]  # blk0 wu cross
        s_wd0 = en(nc.semaphore(name="s_wd0"))  # blk0 wd
        s_w0 = en(nc.semaphore(name="s_w0"))  # even blocks >= 2 (96/blk)
        s_w1 = en(nc.semaphore(name="s_w1"))  # odd blocks (96/blk)
        s_ht = [en(nc.semaphore(name=f"s_ht{j}")) for j in range(SLOTS)]  # h tile slots
        s_yd = [en(nc.semaphore(name=f"s_yd{j}")) for j in range(SLOTS)]  # y store slots
        s_g = en(nc.semaphore(name="s_g"))  # PE: gate groups done (1/gi)
        s_u = en(nc.semaphore(name="s_u"))  # PE: up groups done (1/gi)
        s_silu = en(nc.semaphore(name="s_silu"))  # ACT: silu into s_sb done (1/gi)
        s_mul = en(nc.semaphore(name="s_mul"))  # DVE: s_sb *= up done (1/gi)
        s_c1 = en(nc.semaphore(name="s_c1"))  # ACT: a1b cast done (1/gi)
        s_a1 = en(nc.semaphore(name="s_a1"))  # ACT: a1 cast done (1/gi)
        s_sub = en(nc.semaphore(name="s_sub"))  # DVE: a2b sub done (1/gi)
        s_down = en(nc.semaphore(name="s_down"))  # PE: down groups done (1/di)
        s_yupd = en(nc.semaphore(name="s_yupd"))  # DVE: y copy done (1/di)

        block = en(nc.Block())

        # ---------------- weight DMA stream (sync engine / HWDGE) --------
        @block.sync
        def _(sync):
            for bs in range(NB):
                b = bs % NWB  # slice index into the weight blobs
                buf = bs % 2
                if bs == 1:
                    # block 1 isn't needed until its first pass; keep its
                    # DMA out of the contended startup window
                    sync.wait_ge(s_h0, 16)
                    sync.wait_ge(s_h0b, 16)
                    for j in range(1, NCT[0]):
                        sync.wait_ge(s_ht[j], 16 * ht_cnt[0][j])
                if bs >= 2:
                    # WAR: buffer bs%2 still read by block bs-2's gus/downs
                    sync.wait_ge(s_down, 8 * blk_ctg_end[bs - 2])
                if bs == 0:
                    # (h ct0 is issued from the DVE queue, in parallel with
                    # these weight pieces — the sync queue is feed-limited
                    # at startup)
                    for ft in range(FT_PER):
                        sync.dma_start(
                            wgm_sb[:, buf, ft], wgm_v[:, b, ft]
                        ).then_inc(s_pg[ft], 16)
                        sync.dma_start(
                            wgc_sb[:, buf, ft], wgc_v[:, b, ft]
                        ).then_inc(s_pgc[ft], 16)
                        sync.dma_start(
                            wum_sb[:, buf, ft], wum_v[:, b, ft]
                        ).then_inc(s_pu[ft], 16)
                        sync.dma_start(
                            wuc_sb[:, buf, ft], wuc_v[:, b, ft]
                        ).then_inc(s_puc[ft], 16)
                    if NCT[0] >= 2:
                        # wd isn't needed until the first down; release its
                        # generation once the last up piece is in — h ct1's
                        # descriptors (launched off s_puc[1]) are already
                        # queued ahead of it
                        sync.wait_ge(s_puc[FT_PER - 1], 16)
                    sync.dma_start(wdm_sb[:, buf], wdm_v[:, b]).then_inc(s_wd0, 16)
                    sync.dma_start(wdc_sb[:, buf], wdc_v[:, b]).then_inc(s_wd0, 16)
                else:
                    sw = s_w1 if bs % 2 == 1 else s_w0
                    sync.dma_start(wgm_sb[:, buf], wgm_v[:, b]).then_inc(sw, 16)
                    sync.dma_start(wgc_sb[:, buf], wgc_v[:, b]).then_inc(sw, 16)
                    sync.dma_start(wum_sb[:, buf], wum_v[:, b]).then_inc(sw, 16)
                    sync.dma_start(wuc_sb[:, buf], wuc_v[:, b]).then_inc(sw, 16)
                    sync.dma_start(wdm_sb[:, buf], wdm_v[:, b]).then_inc(sw, 16)
                    sync.dma_start(wdc_sb[:, buf], wdc_v[:, b]).then_inc(sw, 16)
            p = NP - 1
            for j in range(NCT[p]):
                ctg = ctg_base[p] + j
                coff = tile_offs[p][j]
                ctw = tiles[p][j]
                tsl = slice(pass_tok0[p] + coff, pass_tok0[p] + coff + ctw)
                for hp in range(4):
                    sync.wait_ge(s_yupd, YUPD_N * ctg + YUPD_FREE[2 * hp + 1])
                    # s_yf, not s_yd: the SWDGE ring owns the s_yd sems,
                    # and nothing waits on the final pass's stores anyway
                    sync.dma_start(
                        yT_v[:, 2 * hp : 2 * hp + 2, tsl],
                        y_sb[:, 2 * hp : 2 * hp + 2, coff : coff + ctw],
                    ).then_inc(s_yf, 16)

        # ---------------- hT loads + y stores (gpsimd / SWDGE) -----------
        @block.gpsimd
        def _(gp):
            def load_h(p):
                # tile 0 of pass p>=1 goes to the h_pre prefetch buffer,
                # issued as soon as pass p-1's first gu released it
                if p >= 1:
                    gp.wait_ge(s_u, 4 * (ctg_base[p - 1] + 1))
                    w0 = tiles[p][0]
                    tsl = slice(pass_tok0[p], pass_tok0[p] + w0)
                    gp.dma_start(h_pre[:, :, :, :w0], hT_v[:, :, :, tsl]).then_inc(
                        s_ht[0], 16
                    )
                off = 0
                for i, wdt in enumerate(tiles[p]):
                    if i == 0:
                        off += wdt
                        continue
                    if p == 0:
                        # startup: the DMA engines serialize, so keep the
                        # 1MB h tiles out of the block-0 weight feed. The
                        # SWDGE launch latency is ~4us, so release tile1's
                        # GENERATION while the ft2/ft3 weight pieces still
                        # stream — its transfer then enters the queue right
                        # behind uc3 instead of 4us late.
                        if i == 1:
                            gp.wait_ge(s_puc[1], 16)
                        elif i == 2:
                            gp.wait_ge(s_wd0, 32)
                    if p >= 1:
                        # WAR on h_sb cols [off, off+wdt): last readers are
                        # pass p-1's gus of the overlapping tiles
                        m = max(_overlaps(p - 1, off, off + wdt), default=-1)
                        gp.wait_ge(s_u, 4 * (ctg_base[p - 1] + m + 1))
                    tsl = slice(pass_tok0[p] + off, pass_tok0[p] + off + wdt)
                    gp.dma_start(
                        h_sb[:, :, :, off : off + wdt], hT_v[:, :, :, tsl]
                    ).then_inc(s_ht[i], 16)
                    off += wdt

            def store_y_tiles(p):
                # stream each finished ct tile out as soon as its yupds
                # land; the last pass goes out in ht-pair pieces
                for j in range(NCT[p]):
                    ctg = ctg_base[p] + j
                    coff = tile_offs[p][j]
                    ctw = tiles[p][j]
                    tsl = slice(pass_tok0[p] + coff, pass_tok0[p] + coff + ctw)
                    if p == NP - 1:
                        for hp in range(4):
                            # hts [2hp, 2hp+2) are covered by yupd op
                            # YUPD_FREE[2hp+1] of this ctg
                            gp.wait_ge(s_yupd, YUPD_N * ctg + YUPD_FREE[2 * hp + 1])
                            gp.dma_start(
                                yT_v[:, 2 * hp : 2 * hp + 2, tsl],
                                y_sb[:, 2 * hp : 2 * hp + 2, coff : coff + ctw],
                            ).then_inc(s_yd[j], 16)
                    else:
                        gp.wait_ge(s_yupd, YUPD_N * (ctg + 1))
                        gp.dma_start(
                            yT_v[:, :, tsl], y_sb[:, :, coff : coff + ctw]
                        ).then_inc(s_yd[j], 16)

            load_h(0)
            if NP > 1:
                load_h(1)
            for p in range(2, NP):
                store_y_tiles(p - 2)
                load_h(p)
            for p in range(max(NP - 2, 0), NP - 1):
                store_y_tiles(p)
            # the FINAL pass's stores are issued from the sync queue:
            # HWDGE generation (0.62us) beats SWDGE (1.1us) on the
            # end-of-run critical path, and sync is idle by then

        # ---------------- PE stream (one ct-tile lookahead) ----------------
        @block.tensor
        def _(te):
            def gu(ctg):
                p, ct, ctw, coff = ctg_pfc[ctg]
                bs = pass_wb[p]
                buf = bs % 2
                if p == 0 and ct == 0:
                    te.wait_ge(s_h0, 16)
                elif ct == 0:
                    te.wait_ge(s_ht[0], 16 * (ht_cnt[p][0] - 1))
                else:
                    te.wait_ge(s_ht[ct], 16 * ht_cnt[p][ct])
                if ct == 0 and bs > 0 and p == blk_pass_first[bs]:
                    te.wait_ge(s_w1 if bs % 2 == 1 else s_w0, swp_need(bs))
                use_pre = p >= 1 and ct == 0
                csl = slice(coff, coff + ctw)

                def rhs_main(kp):
                    if use_pre:
                        return h_pre[:, 2 * kp : 2 * kp + 2, 1, :ctw]
                    return h_sb[:, 2 * kp : 2 * kp + 2, 1, csl]

                def rhs_cross(k):
                    if use_pre:
                        return h_pre[:, k, :, :ctw]
                    return h_sb[:, k, :, csl]

                blk0 = p == 0 and ct == 0
                for ft in range(FT_PER):
                    gi = ctg * 4 + ft
                    gb = gi % 2
                    if blk0:
                        # startup: mains need only the hi h plane + the gm
                        # piece; the cross wait sits MID-GROUP so the PE
                        # starts ~3us before the gc piece lands
                        te.wait_ge(s_pg[ft], 16)
                    if gi >= 2:
                        te.wait_ge(s_silu, gi - 1)
                    for kp in range(KP):
                        nc.tensor.matmul(
                            g_ps[gb][:, :ctw],
                            wgm_sb[:, buf, ft, kp],
                            rhs_main(kp),
                            start=(kp == 0),
                            stop=False,
                            perf_mode=DR,
                        )
                    if blk0:
                        warm(WARM_PAD)
                        if ft == 0:
                            te.wait_ge(s_h0b, 16)
                        te.wait_ge(s_pgc[ft], 16)
                    for k in range(KT):
                        mm = nc.tensor.matmul(
                            g_ps[gb][:, :ctw],
                            wgc_sb[:, buf, ft, k],
                            rhs_cross(k),
                            start=False,
                            stop=(k == KT - 1),
                            perf_mode=DR,
                        )
                        if k == KT - 1:
                            mm.then_inc(s_g, 1)
                    if blk0:
                        warm(WARM_PAD)
                        te.wait_ge(s_pu[ft], 16)
                    if gi >= 2:
                        te.wait_ge(s_mul, gi - 1)
                    for kp in range(KP):
                        nc.tensor.matmul(
                            u_ps[gb][:, :ctw],
                            wum_sb[:, buf, ft, kp],
                            rhs_main(kp),
                            start=(kp == 0),
                            stop=False,
                            perf_mode=DR,
                        )
                    if blk0:
                        te.wait_ge(s_puc[ft], 16)
                    for k in range(KT):
                        mm = nc.tensor.matmul(
                            u_ps[gb][:, :ctw],
                            wuc_sb[:, buf, ft, k],
                            rhs_cross(k),
                            start=False,
                            stop=(k == KT - 1),
                            perf_mode=DR,
                        )
                        if k == KT - 1:
                            mm.then_inc(s_u, 1)

            def down(ctg):
                p, ct, ctw, coff = ctg_pfc[ctg]
                bs = pass_wb[p]
                buf = bs % 2
                ab = ctg % 2
                if ct == 0 and p == blk_pass_first[bs]:
                    if bs == 0:
                        te.wait_ge(s_wd0, 32)
                    else:
                        te.wait_ge(s_w1 if bs % 2 == 1 else s_w0, swp_need(bs))
                te.wait_ge(s_sub, 4 * (ctg + 1))
                te.wait_ge(s_a1, 4 * (ctg + 1))
                for ht in range(HT):
                    di = ctg * 8 + ht
                    db = di % 4
                    if di >= 4:
                        # bank db last written by down di-4; s_yupd counts
                        # the 5 copy-ops per ctg (see YUPD_FREE)
                        cp, hp = (di - 4) // 8, (di - 4) % 8
                        te.wait_ge(s_yupd, YUPD_N * cp + YUPD_FREE[hp])
                    for fp in range(FPAIR):
                        nc.tensor.matmul(
                            yp_ps[:, db, :ctw],
                            wdm_sb[:, buf, ht, fp],
                            act8[:, ab, 2 * fp : 2 * fp + 2, 0, :ctw],
                            start=(fp == 0),
                            stop=False,
                            perf_mode=DR,
                        )
                    for ft in range(FT_PER):
                        mm = nc.tensor.matmul(
                            yp_ps[:, db, :ctw],
                            wdc_sb[:, buf, ht, ft],
                            act8[:, ab, ft, 1:3, :ctw],
                            start=False,
                            stop=(ft == FT_PER - 1),
                            perf_mode=DR,
                        )
                        if ft == FT_PER - 1:
                            mm.then_inc(s_down, 1)

            def warm(n):
                # p-state keep-alive: zero matmuls into yp bank 0 (reset by
                # down(0)'s start=True later). Any PE idle gap drops the
                # clock to 1.2GHz for 3us; these span the startup DMA waits
                # so the real stream starts and stays at 2.4GHz.
                for _ in range(n):
                    nc.tensor.matmul(
                        yp_ps[:, 0, :512],
                        warm_sb[:, :, :128],
                        warm_sb[:, :, :],
                        start=True,
                        stop=True,
                        perf_mode=DR,
                    )

            te.wait_ge(s_warm, 1)
            warm(WARM_N1)
            gu(0)
            for ctg in range(TOTAL_CT):
                if ctg + 1 < TOTAL_CT:
                    gu(ctg + 1)
                down(ctg)

        # ---------------- ACT stream (silu + fp8 casts) ------------------
        @block.scalar
        def _(sc):
            def silu_one(ctg, ft):
                ab = ctg % 2
                ctw = ctg_pfc[ctg][2]
                gi = ctg * 4 + ft
                gb = gi % 2
                if ft == 0 and ctg >= 2:
                    # WAR on s_sb[ab]: DVE subs of ctg-2 done
                    sc.wait_ge(s_sub, 4 * (ctg - 1))
                sc.wait_ge(s_g, gi + 1)
                nc.scalar.activation(
                    s_sb[:, ab, ft, :ctw],
                    g_ps[gb][:, :ctw],
                    ACT_FUNC,
                    scale=1.0 / SC,
                ).then_inc(s_silu, 1)

            def casts_one(ctg, ft):
                ab = ctg % 2
                ctw = ctg_pfc[ctg][2]
                gi = ctg * 4 + ft
                if ft == 0 and ctg >= 2:
                    # WAR on act8[ab]: down mms of ctg-2 done
                    sc.wait_ge(s_down, 8 * (ctg - 1))
                sc.wait_ge(s_mul, gi + 1)
                nc.scalar.activation(
                    act8[:, ab, ft, 0, :ctw],
                    s_sb[:, ab, ft, :ctw],
                    mybir.ActivationFunctionType.Copy,
                ).then_inc(s_c1, 1)
                nc.scalar.activation(
                    act8[:, ab, ft, 2, :ctw],
                    s_sb[:, ab, ft, :ctw],
                    mybir.ActivationFunctionType.Copy,
                    scale=0.125,
                ).then_inc(s_a1, 1)

            # h ct0 from the ACT engine's HWDGE queue: runs in parallel
            # with the sync queue's block-0 weight pieces (startup is
            # feed-limited; ACT's first silu is ~4us in anyway). Split by
            # plane: the HI plane (v=1) is all the main-term DRs need, so
            # the first matmul isn't gated on the full 1MB tile.
            w0 = tiles[0][0]
            sc.dma_start(h_sb[:, :, 1, :w0], hT_v[:, :, 1, :w0]).then_inc(s_h0, 16)
            sc.dma_start(h_sb[:, :, 0, :w0], hT_v[:, :, 0, :w0]).then_inc(s_h0b, 16)

            def yupd_act(ctg):
                # tail-only: ctg T-2's y copies on ACT (DVE is congested
                # with muls/subs there and the back-to-back final downs
                # would stall ~4us on the psum-bank WAR)
                p, ct, ctw, coff = ctg_pfc[ctg]
                csl = slice(coff, coff + ctw)
                for j, (h0, h1) in enumerate(YUPD_HT):
                    sc.wait_ge(s_down, ctg * 8 + h1)
                    if j == 0:
                        # cross-engine s_yupd handover: prove all DVE
                        # increments (through ctg-1) have landed
                        sc.wait_ge(s_yupd, YUPD_N * ctg)
                    if j == 0 and p > 0:
                        ov = _overlaps(p - 1, coff, coff + ctw) or range(SLOTS)
                        for i in ov:
                            sc.wait_ge(s_yd[i], 16 * yd_cnt[p - 1][i])
                    nc.scalar.activation(
                        y_sb[:, h0:h1, csl],
                        yp_ps[:, h0 % 4 : (h1 - 1) % 4 + 1, :ctw],
                        mybir.ActivationFunctionType.Copy,
                        scale=1.0 / SC,
                    ).then_inc(s_yupd, 1)

            for ctg in range(TOTAL_CT):
                # interleave: casts lag silus by one ft so ACT never waits
                # long on the DVE muls
                silu_one(ctg, 0)
                silu_one(ctg, 1)
                casts_one(ctg, 0)
                silu_one(ctg, 2)
                casts_one(ctg, 1)
                silu_one(ctg, 3)
                casts_one(ctg, 2)
                casts_one(ctg, 3)
            if TOTAL_CT >= 2:
                yupd_act(TOTAL_CT - 2)

        # ---------------- DVE stream (mul + sub + y copy) ------------------
        @block.vector
        def _(ve):
            def mul_one(ctg, ft):
                ab = ctg % 2
                ctw = ctg_pfc[ctg][2]
                gi = ctg * 4 + ft
                gb = gi % 2
                ve.wait_ge(s_silu, gi + 1)
                ve.wait_ge(s_u, gi + 1)
                nc.vector.tensor_mul(
                    s_sb[:, ab, ft, :ctw],
                    s_sb[:, ab, ft, :ctw],
                    u_ps[gb][:, :ctw],
                ).then_inc(s_mul, 1)

            def sub_one(ctg, ft):
                ab = ctg % 2
                ctw = ctg_pfc[ctg][2]
                gi = ctg * 4 + ft
                ve.wait_ge(s_c1, gi + 1)
                nc.vector.tensor_sub(
                    act8[:, ab, ft, 1, :ctw],
                    s_sb[:, ab, ft, :ctw],
                    act8[:, ab, ft, 0, :ctw],
                ).then_inc(s_sub, 1)

            def muls(ctg):
                for ft in range(FT_PER):
                    mul_one(ctg, ft)

            def subs(ctg):
                for ft in range(FT_PER):
                    sub_one(ctg, ft)

            def yupd(ctg):
                p, ct, ctw, coff = ctg_pfc[ctg]
                csl = slice(coff, coff + ctw)
                for j, (h0, h1) in enumerate(YUPD_HT):
                    ve.wait_ge(s_down, ctg * 8 + h1)
                    if j == 0 and TOTAL_CT >= 2 and ctg == TOTAL_CT - 1:
                        # cross-engine s_yupd handover: prove ACT's T-2
                        # increments have landed before DVE adds its own
                        ve.wait_ge(s_yupd, YUPD_N * ctg)
                    if j == 0 and p > 0:
                        # WAR on y_sb cols: stores of ALL overlapping tiles
                        # through pass p-1 must have drained
                        ov = _overlaps(p - 1, coff, coff + ctw) or range(SLOTS)
                        for i in ov:
                            ve.wait_ge(s_yd[i], 16 * yd_cnt[p - 1][i])
                    nc.vector.tensor_scalar_mul(
                        y_sb[:, h0:h1, csl],
                        yp_ps[:, h0 % 4 : (h1 - 1) % 4 + 1, :ctw],
                        1.0 / SC,
                    ).then_inc(s_yupd, 1)

            # Order per steady-state step c: [muls(c+1), yupd(c), subs(c+1)].
            # yupd(c) must start the moment down(c) begins on the PE (its
            # copies gate down(c)'s later hts via the 4-bank yp WAR), so it
            # cannot sit behind subs(c+1) whose casts depend on gu(c+1)'s
            # end. subs(c) are issued in step c-1, before yupd(c-1) would
            # block on down(c-1) -> no engine-order deadlock.
            nc.vector.memset(warm_sb[:, :, :], 0).then_inc(s_warm, 1)
            muls(0)
            subs(0)
            for ctg in range(TOTAL_CT):
                if ctg + 1 < TOTAL_CT:
                    muls(ctg + 1)
                if not (TOTAL_CT >= 2 and ctg == TOTAL_CT - 2):
                    yupd(ctg)  # ctg T-2's copies run on ACT instead
                if ctg + 1 < TOTAL_CT:
                    subs(ctg + 1)

    return nc


# ----------------------------------------------------------------------------
# Host side
# ----------------------------------------------------------------------------


def _q8(x):
    """fp8 e4m3 quantize (round-to-nearest-even), back to f32."""
    return x.astype(E4M3).astype(np.float32)


def _q8r(x):
    """fp8 e4m3 quantize, keep fp8 dtype."""
    return np.ascontiguousarray(x.astype(E4M3))


def _route(h, Wr, topk):
    """Exact fp32 replica of the reference router. Returns sel [T,k], w [T,k]."""
    logits = h @ Wr.T  # [T, E]
    logits = logits.astype(np.float32)
    m = logits.max(axis=-1, keepdims=True)
    e = np.exp(logits - m)
    p = e / e.sum(axis=-1, keepdims=True)
    sel = np.argsort(-p, axis=-1, kind="stable")[:, :topk]  # ties -> lower idx
    w = np.take_along_axis(p, sel, axis=-1)
    if topk != 1:
        w = w / w.sum(axis=-1, keepdims=True)
    return sel, w.astype(np.float32)


def _pack_gu(WT):
    """Gate/up weight packing. WT: [H, FSH] f32 (already pre-scaled).
    Returns (main, cross) fp8 blobs shaped [128, FT*KP*2*128] and
    [128, FT*KT*2*128]."""
    W1 = _q8(WT)
    W1s = W1 * SC  # exact in fp8 (power-of-two, no overflow)
    W2s = _q8(SC * (WT - W1))
    # main: [p, ft, kp, i, fi] = W1s[(2kp+i)*128+p, ft*128+fi]
    main = W1s.reshape(KP, 2, 128, FT_PER, 128).transpose(2, 3, 0, 1, 4)
    # cross: [p, ft, k, v, fi] with v0 = W1 (hi, unscaled), v1 = W2s
    cross = np.stack([W1, W2s]).reshape(2, KT, 128, FT_PER, 128).transpose(
        2, 3, 1, 0, 4
    )
    return (
        _q8r(main).reshape(128, -1),
        _q8r(cross).reshape(128, -1),
    )


def _pack_d(DT):
    """Down weight packing. DT: [FSH, H] f32 (pre-scaled by 4).
    Returns (main, cross) fp8 blobs [128, HT*FPAIR*2*128], [128, HT*FT*2*128]."""
    D1 = _q8(DT)
    D2s = _q8(8.0 * (DT - D1))
    # main: [p, ht, fp, i, hi] = D1[(2fp+i)*128+p, ht*128+hi]
    main = D1.reshape(FPAIR, 2, 128, HT, 128).transpose(2, 3, 0, 1, 4)
    # cross: [p, ht, ft, v, hi] with v0 = D1 (hi), v1 = D2s
    cross = np.stack([D1, D2s]).reshape(2, FT_PER, 128, HT, 128).transpose(
        2, 3, 1, 0, 4
    )
    return (
        _q8r(main).reshape(128, -1),
        _q8r(cross).reshape(128, -1),
    )


def kernel(x, Wr, Wg, Wu, Wd, topk):
    topk = int(topk)
    x = np.asarray(x, dtype=np.float32)
    Wr = np.asarray(Wr, dtype=np.float32)
    Wg = np.asarray(Wg, dtype=np.float32)
    Wu = np.asarray(Wu, dtype=np.float32)
    Wd = np.asarray(Wd, dtype=np.float32)

    T = x.shape[0] * x.shape[1]
    h = np.ascontiguousarray(x.reshape(T, H))

    sel, w = _route(h, Wr, topk)

    idx = [None] * E
    wts = [None] * E
    for e in range(E):
        tok, kk = np.nonzero(sel == e)
        idx[e] = tok
        wts[e] = w[tok, kk]
    counts = [len(i) for i in idx]

    present = [e for e in range(E) if counts[e] > 0]
    sizes = [max(256, ((counts[e] + 31) // 32) * 32) for e in present]
    TC = sum(sizes)
    tok0 = [sum(sizes[:i]) for i in range(len(sizes))]

    nc = build_program(sizes)

    # hT: all experts' tokens grouped and padded — identical on every core.
    # fp8 split: v1 = h1 = fp8(h), v0 = h2s = fp8(32*(h-h1)).
    hTfull = h.T  # [H, T] view
    hT = np.zeros((H, TC), dtype=np.float32)
    for i, e in enumerate(present):
        hT[:, tok0[i] : tok0[i] + counts[e]] = hTfull[:, idx[e]]
    h1 = _q8(hT)
    h2s = SC * (hT - h1)
    hpack = np.empty((KT, 2, 128, TC), dtype=E4M3)
    hpack[:, 0] = h2s.reshape(KT, 128, TC).astype(E4M3)
    hpack[:, 1] = h1.reshape(KT, 128, TC).astype(E4M3)
    hT_in = np.ascontiguousarray(hpack).reshape(KT * 2 * 128, TC)

    # per-core weight slices: core k owns F rows [k*FSH,(k+1)*FSH) of every
    # expert, packed into fp8 main/cross blobs (concatenated in block order)
    NWB = len(present)
    in_maps = []
    for k in range(E):
        fs = slice(k * FSH, (k + 1) * FSH)
        gm = np.empty((128, NWB, FT_PER * KP * 2 * 128), dtype=E4M3)
        gc = np.empty((128, NWB, FT_PER * KT * 2 * 128), dtype=E4M3)
        um = np.empty((128, NWB, FT_PER * KP * 2 * 128), dtype=E4M3)
        uc = np.empty((128, NWB, FT_PER * KT * 2 * 128), dtype=E4M3)
        dm = np.empty((128, NWB, HT * FPAIR * 2 * 128), dtype=E4M3)
        dc = np.empty((128, NWB, HT * FT_PER * 2 * 128), dtype=E4M3)
        for i, e in enumerate(present):
            gm[:, i], gc[:, i] = _pack_gu(np.ascontiguousarray(Wg[e, fs, :].T))
            um[:, i], uc[:, i] = _pack_gu(
                np.ascontiguousarray(Wu[e, fs, :].T) * 0.25
            )
            dm[:, i], dc[:, i] = _pack_d(
                np.ascontiguousarray(Wd[e, :, fs].T) * 4.0
            )
        in_maps.append(
            {
                "hT": hT_in,
                "wgm": gm.reshape(128, -1),
                "wgc": gc.reshape(128, -1),
                "wum": um.reshape(128, -1),
                "wuc": uc.reshape(128, -1),
                "wdm": dm.reshape(128, -1),
                "wdc": dc.reshape(128, -1),
            }
        )

    res = run_bass_kernel_spmd(nc, in_maps, core_ids=list(range(E)))

    # sum the 8 partial projections (bf16 -> f32), then combine
    ysum = res.results[0]["yT"].astype(np.float64)
    for k in range(1, E):
        ysum += res.results[k]["yT"].astype(np.float64)
    out = np.zeros((T, H), dtype=np.float32)
    for i, e in enumerate(present):
        cnt = counts[e]
        ye = ysum[:, tok0[i] : tok0[i] + cnt].T  # [cnt, H] f64
        out[idx[e]] += (wts[e][:, None].astype(np.float64) * ye).astype(np.float32)
    return out.reshape(x.shape)


# revision 63
# speedup vs baseline: 1.0012x; 1.0012x over previous
"""MoE MLP (Mixtral-style top-2 routing) on 8 Trainium2 NeuronCores.

Strategy: expert-COLUMN-parallel (F-sharding) with fp8 split-compensation
matmuls. The router (tiny: T x H x E) runs on host in fp32, exactly
mirroring the reference math. Tokens are grouped by expert on host; EVERY
core processes EVERY expert's token group, but only a 512-wide slice of
the F dimension (core k owns F rows [k*512,(k+1)*512) of each expert's
Wg/Wu and the matching Wd columns). Per-core work is identical by
construction. Each core produces a PARTIAL down projection; the host sums
the 8 partials and applies the top-k combine weights in a weighted
scatter-add.

fp8 split-compensation: every operand X (weights on host, h on host, the
activation a on device) is represented as X = X1 + X2/S with X1 = fp8(X),
X2 = fp8(S*(X - X1)), S = 32 (16 for the a-residual; power-of-two scales
are exact in fp8). A logical 128-contraction matmul A^T B then needs the
three products A1B1 + A1B2 + A2B1 (the lo*lo term is ~0.13% of a percent
and dropped). DoubleRow fp8 matmuls compute TWO independent 128-
contraction products per instruction at 0.5 cycles/column, so per k-tile
the split costs 1.5 DR slots = 0.75x the f32r time:
  main DR (per k-PAIR):  (S*A1[k], S*A1[k+1]) x (B1[k], B1[k+1])
  cross DR (per k-tile): (A1[k], S*A2[k])     x (S*B2[k], B1[k])
All terms land uniformly scaled by S in the PSUM group. Scale bookkeeping:
  gate psum = 32*g   (silu reads it with ACT scale=1/32)
  up weights pre-scaled by 1/4 on host -> up psum = 8*u
  a_bar = silu(g) * u_psum = 8*a  (DVE mul, scale comes for free)
  a1b = fp8(a_bar) = 8*a1;  a2b = fp8(a_bar - a1b) = 8*(a-a1)  [DVE sub]
  a1  = fp8(a_bar * 1/8)  [ACT copy, scale=0.125]
  down weights pre-scaled by 4 on host -> down psum = 32*y
  y_sb = psum * 1/32 (DVE tensor_scalar_mul), stored bf16
End-to-end quantization error ~0.3-0.4% (budget 2e-2).

Program: one weight BLOCK per expert (double-buffered); each expert's
tokens form one pass (<=2304 resident tokens, next pass's first h tile
prefetched via h_pre); per pass a loop over near-equal <=512-wide ct
tiles. The PE stream runs one ct-tile ahead of the silu/mul/split stage;
the DVE stream orders [muls(c+1), y-copies(c), subs(c+1)] so the psum
bank copies start the moment down(c) begins; y-copies go out singles-
then-pairs to stay ahead of the PE's 0.64us/ht down cadence.
"""

import numpy as np
import ml_dtypes
import concourse.bass as bass
import concourse.mybir as mybir
from concourse.bass_utils import run_bass_kernel_spmd

f32 = mybir.dt.float32
fp8 = mybir.dt.float8e4
bf16 = mybir.dt.bfloat16
E4M3 = ml_dtypes.float8_e4m3fn
ACT_FUNC = mybir.ActivationFunctionType.Silu  # swapped in coresim_check.py

B, S, H, F, E = 4, 2048, 1024, 4096, 8
KT = H // 128  # 8 k-tiles of the H contraction
KP = KT // 2  # 4 k-pairs for the main-term DRs
FSH = F // 8  # 512-wide per-core F slice
FT_PER = FSH // 128  # 4 f-tiles per slice
FPAIR = FT_PER // 2  # 2 f-pairs for the down main-term DRs
HT = H // 128  # 8 output H tiles
CT_W = 512  # max token tile width (moving dim N)
PASS_MAX = 2304  # SBUF budget for h_sb/y_sb columns (one pass per expert)
SC = 32.0  # residual scale (power of two)
DR = mybir.MatmulPerfMode.DoubleRow
WARM_N1 = 13  # dummy matmuls spanning the pre-first-weight DMA wait
WARM_PAD = 0  # dummies padding ctg0 feed stalls (0 = disabled)


def _split_tiles(pass_size, lead256=False):
    """Split a pass into near-equal ct tiles (<=512 wide, multiples of 32):
    equal widths avoid narrow trailing tiles whose short gu window can't
    hide the act-chain latency. lead256: a 256 first tile (pass 0 only)
    shrinks the h DMA on the startup critical path."""
    widths = []
    rest = pass_size
    if lead256 and rest >= 1024:
        widths.append(256)
        rest -= 256
    n = -(-rest // 512)
    base = (rest // n) // 32 * 32
    ws = [base] * n
    rem = (rest - base * n) // 32
    for i in range(rem):
        ws[i] += 32
    widths += ws
    assert sum(widths) == pass_size and all(256 <= w <= 512 for w in widths), widths
    return widths


def _expert_passes(ce_pad):
    """Split one expert's padded token count into passes of <=PASS_MAX,
    near-equal, multiples of 32."""
    n = -(-ce_pad // PASS_MAX)
    base = (ce_pad // n) // 32 * 32
    out = [base] * n
    rem = (ce_pad - base * n) // 32
    for i in range(rem):
        out[i] += 32
    assert sum(out) == ce_pad and all(256 <= ps <= PASS_MAX for ps in out), out
    return out


def build_program(expert_sizes, repeat=1):
    """Per-core Bass program. `expert_sizes`: padded token count per
    present expert (in block order). Each expert is one weight block
    spanning 1+ token passes. `repeat` re-runs everything (bench only)."""
    NWB = len(expert_sizes)
    pass_sizes = []
    pass_wb = []  # weight block (expert slot) per pass
    for b, ce in enumerate(expert_sizes):
        for ps in _expert_passes(ce):
            pass_sizes.append(ps)
            pass_wb.append(b)
    TC = sum(pass_sizes)
    tok0 = [sum(pass_sizes[:p]) for p in range(len(pass_sizes))]

    pass_sizes = pass_sizes * repeat
    pass_tok0 = tok0 * repeat
    pass_wb = [b + r * NWB for r in range(repeat) for b in pass_wb]
    NP = len(pass_sizes)
    NB = NWB * repeat  # global weight-block sequence length
    PSMAX = max(pass_sizes)
    tiles = [_split_tiles(ps) for ps in pass_sizes]
    NCT = [len(t) for t in tiles]
    tile_offs = [[sum(tiles[p][:i]) for i in range(NCT[p])] for p in range(NP)]

    # ctg enumeration: for p, for ct -> (p, ct, width, offset)
    ctg_base = [0] * (NP + 1)
    for p in range(NP):
        ctg_base[p + 1] = ctg_base[p] + NCT[p]
    TOTAL_CT = ctg_base[NP]
    ctg_pfc = []
    for p in range(NP):
        for ct in range(NCT[p]):
            ctg_pfc.append((p, ct, tiles[p][ct], tile_offs[p][ct]))

    # last ctg (exclusive) of each weight block
    blk_pass_last = {}
    for p in range(NP):
        blk_pass_last[pass_wb[p]] = p
    blk_ctg_end = {b: ctg_base[blk_pass_last[b] + 1] for b in blk_pass_last}
    blk_pass_first = {}
    for p in range(NP - 1, -1, -1):
        blk_pass_first[pass_wb[p]] = p

    SLOTS = max(NCT)

    # yupd op layout per ctg: hts [0],[1],[2],[3],[4,5],[6,7] — singles
    # while the PE's down still races ahead on fresh banks (each single
    # frees its bank ~0.8us before the down's reuse), then pairs for
    # throughput (1.26us per 2 ht < the PE's 1.28us cadence).
    YUPD_HT = [(0, 1), (1, 2), (2, 3), (3, 4), (4, 6), (6, 8)]
    # op index (1-based, within a ctg) that frees each ht's psum bank
    YUPD_FREE = {0: 1, 1: 2, 2: 3, 3: 4, 4: 5, 5: 5, 6: 6, 7: 6}
    YUPD_N = len(YUPD_HT)

    # Per-tile-SLOT h DMA counts: slot j of pass p has been loaded
    # ht_cnt[p][j] times through pass p (each slot has its own semaphore).
    ht_cnt = []
    cnt = [0] * SLOTS
    for p in range(NP):
        for j in range(NCT[p]):
            cnt[j] += 1
        ht_cnt.append(list(cnt))

    # y store counts per slot, same scheme
    yd_cnt = []
    cnt = [0] * SLOTS
    for p in range(NP):
        for j in range(NCT[p]):
            cnt[j] += 4 if p == NP - 1 else 1
        yd_cnt.append(list(cnt))

    def _overlaps(p, lo, hi):
        """Tile indices of pass p whose column range intersects [lo, hi)."""
        return [
            i
            for i, (o, w) in enumerate(zip(tile_offs[p], tiles[p]))
            if o < hi and o + w > lo
        ]

    # Weight-block thresholds: block 0 is piecewise on dedicated sems
    # (s_pg/s_pu per ft + s_wd0); blocks >= 1 alternate parity sems
    # s_w0/s_w1 (+96 each: 6 DMAs x 16).
    def swp_need(bs):
        assert bs >= 1
        n_parity = (bs + 1) // 2 if bs % 2 == 1 else bs // 2
        return 96 * n_parity

    nc = bass.Bass()
    # h: interleaved residual pairs, row = k*256 + v*128 + p (v0 = 32*h_lo,
    # v1 = h_hi), fp8
    hT = nc.declare_dram_parameter("hT", [KT * 2 * 128, TC], fp8, isOutput=False)
    # weights: partition-major blobs [128, blocks * piece]
    wgm = nc.declare_dram_parameter("wgm", [128, NWB * FT_PER * KP * 2 * 128], fp8, isOutput=False)
    wgc = nc.declare_dram_parameter("wgc", [128, NWB * FT_PER * KT * 2 * 128], fp8, isOutput=False)
    wum = nc.declare_dram_parameter("wum", [128, NWB * FT_PER * KP * 2 * 128], fp8, isOutput=False)
    wuc = nc.declare_dram_parameter("wuc", [128, NWB * FT_PER * KT * 2 * 128], fp8, isOutput=False)
    wdm = nc.declare_dram_parameter("wdm", [128, NWB * HT * FPAIR * 2 * 128], fp8, isOutput=False)
    wdc = nc.declare_dram_parameter("wdc", [128, NWB * HT * FT_PER * 2 * 128], fp8, isOutput=False)
    yT = nc.declare_dram_parameter("yT", [H, TC], bf16, isOutput=True)

    hT_v = hT.rearrange("(k v p) t -> p k v t", p=128, v=2)  # [128, KT, 2, TC]
    wgm_v = wgm.rearrange("p (b f k v c) -> p b f k v c", b=NWB, f=FT_PER, k=KP, v=2)
    wgc_v = wgc.rearrange("p (b f k v c) -> p b f k v c", b=NWB, f=FT_PER, k=KT, v=2)
    wum_v = wum.rearrange("p (b f k v c) -> p b f k v c", b=NWB, f=FT_PER, k=KP, v=2)
    wuc_v = wuc.rearrange("p (b f k v c) -> p b f k v c", b=NWB, f=FT_PER, k=KT, v=2)
    wdm_v = wdm.rearrange("p (b j k v c) -> p b j k v c", b=NWB, j=HT, k=FPAIR, v=2)
    wdc_v = wdc.rearrange("p (b j k v c) -> p b j k v c", b=NWB, j=HT, k=FT_PER, v=2)
    yT_v = yT.rearrange("(j p) t -> p j t", p=128)  # [128, HT, TC]

    from contextlib import ExitStack

    with ExitStack() as ctx:
        en = ctx.enter_context
        # h: [part, k, ver, col]; ver0 = 32*h_lo, ver1 = h_hi
        h_sb = en(nc.sbuf_tensor("h_sb", [128, KT, 2, PSMAX], fp8))
        h_pre = en(nc.sbuf_tensor("h_pre", [128, KT, 2, CT_W], fp8))
        y_sb = en(nc.sbuf_tensor("y_sb", [128, HT, PSMAX], bf16))
        wgm_sb = en(nc.sbuf_tensor("wgm_sb", [128, 2, FT_PER, KP, 2, 128], fp8))
        wgc_sb = en(nc.sbuf_tensor("wgc_sb", [128, 2, FT_PER, KT, 2, 128], fp8))
        wum_sb = en(nc.sbuf_tensor("wum_sb", [128, 2, FT_PER, KP, 2, 128], fp8))
        wuc_sb = en(nc.sbuf_tensor("wuc_sb", [128, 2, FT_PER, KT, 2, 128], fp8))
        wdm_sb = en(nc.sbuf_tensor("wdm_sb", [128, 2, HT, FPAIR, 2, 128], fp8))
        wdc_sb = en(nc.sbuf_tensor("wdc_sb", [128, 2, HT, FT_PER, 2, 128], fp8))
        # a_bar = 8*a staging (f32) and the fp8 act versions:
        # ver0 = a1b = 8*a1, ver1 = a2b = 8*(a-a1), ver2 = a1 (unscaled)
        s_sb = en(nc.sbuf_tensor("s_sb", [128, 2, FT_PER, CT_W], f32))
        act8 = en(nc.sbuf_tensor("act8", [128, 2, FT_PER, 3, CT_W], fp8))

        g_ps = [en(nc.psum_tensor(f"g_ps{i}", [128, CT_W], f32)) for i in range(2)]
        u_ps = [en(nc.psum_tensor(f"u_ps{i}", [128, CT_W], f32)) for i in range(2)]
        # one 4-bank tensor so the y copies can go out in ht-PAIRS
        yp_ps = en(nc.psum_tensor("yp_ps", [128, 4, CT_W], f32))

        warm_sb = en(nc.sbuf_tensor("warm_sb", [128, 2, 512], fp8))

        s_warm = en(nc.semaphore(name="s_warm"))  # warm_sb zeroed (DVE memset)
        s_yf = en(nc.semaphore(name="s_yf"))  # final-pass y stores (sync queue)
        s_h0 = en(nc.semaphore(name="s_h0"))  # pass-0 ct0 h, HI plane (v=1)
        s_h0b = en(nc.semaphore(name="s_h0b"))  # pass-0 ct0 h, LO plane (v=0)
        s_pg = [en(nc.semaphore(name=f"s_pg{i}")) for i in range(FT_PER)]  # blk0 wg main
        s_pgc = [en(nc.semaphore(name=f"s_pgc{i}")) for i in range(FT_PER)]  # blk0 wg cross
        s_pu = [en(nc.semaphore(name=f"s_pu{i}")) for i in range(FT_PER)]  # blk0 wu main
        s_puc = [en(nc.semaphore(name=f"s_puc{i}")) for i in range(FT_PER)]  # blk0 wu cross
        s_wd0 = en(nc.semaphore(name="s_wd0"))  # blk0 wd
        s_w0 = en(nc.semaphore(name="s_w0"))  # even blocks >= 2 (96/blk)
        s_w1 = en(nc.semaphore(name="s_w1"))  # odd blocks (96/blk)
        s_ht = [en(nc.semaphore(name=f"s_ht{j}")) for j in range(SLOTS)]  # h tile slots
        s_yd = [en(nc.semaphore(name=f"s_yd{j}")) for j in range(SLOTS)]  # y store slots
        s_g = en(nc.semaphore(name="s_g"))  # PE: gate groups done (1/gi)
        s_u = en(nc.semaphore(name="s_u"))  # PE: up groups done (1/gi)
        s_silu = en(nc.semaphore(name="s_silu"))  # ACT: silu into s_sb done (1/gi)
        s_mul = en(nc.semaphore(name="s_mul"))  # DVE: s_sb *= up done (1/gi)
        s_c1 = en(nc.semaphore(name="s_c1"))  # ACT: a1b cast done (1/gi)
        s_a1 = en(nc.semaphore(name="s_a1"))  # ACT: a1 cast done (1/gi)
        s_sub = en(nc.semaphore(name="s_sub"))  # DVE: a2b sub done (1/gi)
        s_down = en(nc.semaphore(name="s_down"))  # PE: down groups done (1/di)
        s_yupd = en(nc.semaphore(name="s_yupd"))  # DVE: y copy done (1/di)

        block = en(nc.Block())

        # ---------------- weight DMA stream (sync engine / HWDGE) --------
        @block.sync
        def _(sync):
            for bs in range(NB):
                b = bs % NWB  # slice index into the weight blobs
                buf = bs % 2
                if bs == 1:
                    # block 1 isn't needed until its first pass; keep its
                    # DMA out of the contended startup window
                    sync.wait_ge(s_h0, 16)
                    sync.wait_ge(s_h0b, 16)
                    for j in range(1, NCT[0]):
                        sync.wait_ge(s_ht[j], 16 * ht_cnt[0][j])
                if bs >= 2:
                    # WAR: buffer bs%2 still read by block bs-2's gus/downs
                    sync.wait_ge(s_down, 8 * blk_ctg_end[bs - 2])
                if bs == 0:
                    # (h ct0 is issued from the DVE queue, in parallel with
                    # these weight pieces — the sync queue is feed-limited
                    # at startup)
                    for ft in range(FT_PER):
                        sync.dma_start(
                            wgm_sb[:, buf, ft], wgm_v[:, b, ft]
                        ).then_inc(s_pg[ft], 16)
                        sync.dma_start(
                            wgc_sb[:, buf, ft], wgc_v[:, b, ft]
                        ).then_inc(s_pgc[ft], 16)
                        sync.dma_start(
                            wum_sb[:, buf, ft], wum_v[:, b, ft]
                        ).then_inc(s_pu[ft], 16)
                        sync.dma_start(
                            wuc_sb[:, buf, ft], wuc_v[:, b, ft]
                        ).then_inc(s_puc[ft], 16)
                    if NCT[0] >= 2:
                        # wd isn't needed until the first down; release its
                        # generation once the last up piece is in — h ct1's
                        # descriptors (launched off s_puc[1]) are already
                        # queued ahead of it
                        sync.wait_ge(s_puc[FT_PER - 1], 16)
                    sync.dma_start(wdm_sb[:, buf], wdm_v[:, b]).then_inc(s_wd0, 16)
                    sync.dma_start(wdc_sb[:, buf], wdc_v[:, b]).then_inc(s_wd0, 16)
                else:
                    sw = s_w1 if bs % 2 == 1 else s_w0
                    sync.dma_start(wgm_sb[:, buf], wgm_v[:, b]).then_inc(sw, 16)
                    sync.dma_start(wgc_sb[:, buf], wgc_v[:, b]).then_inc(sw, 16)
                    sync.dma_start(wum_sb[:, buf], wum_v[:, b]).then_inc(sw, 16)
                    sync.dma_start(wuc_sb[:, buf], wuc_v[:, b]).then_inc(sw, 16)
                    sync.dma_start(wdm_sb[:, buf], wdm_v[:, b]).then_inc(sw, 16)
                    sync.dma_start(wdc_sb[:, buf], wdc_v[:, b]).then_inc(sw, 16)
            p = NP - 1
            for j in range(NCT[p]):
                ctg = ctg_base[p] + j
                coff = tile_offs[p][j]
                ctw = tiles[p][j]
                tsl = slice(pass_tok0[p] + coff, pass_tok0[p] + coff + ctw)
                for hp in range(4):
                    sync.wait_ge(s_yupd, YUPD_N * ctg + YUPD_FREE[2 * hp + 1])
                    # s_yf, not s_yd: the SWDGE ring owns the s_yd sems,
                    # and nothing waits on the final pass's stores anyway
                    sync.dma_start(
                        yT_v[:, 2 * hp : 2 * hp + 2, tsl],
                        y_sb[:, 2 * hp : 2 * hp + 2, coff : coff + ctw],
                    ).then_inc(s_yf, 16)

        # ---------------- hT loads + y stores (gpsimd / SWDGE) -----------
        @block.gpsimd
        def _(gp):
            def load_h(p):
                # tile 0 of pass p>=1 goes to the h_pre prefetch buffer,
                # issued as soon as pass p-1's first gu released it
                if p >= 1:
                    gp.wait_ge(s_u, 4 * (ctg_base[p - 1] + 1))
                    w0 = tiles[p][0]
                    tsl = slice(pass_tok0[p], pass_tok0[p] + w0)
                    gp.dma_start(h_pre[:, :, :, :w0], hT_v[:, :, :, tsl]).then_inc(
                        s_ht[0], 16
                    )
                off = 0
                for i, wdt in enumerate(tiles[p]):
                    if i == 0:
                        off += wdt
                        continue
                    if p == 0:
                        # startup: the DMA engines serialize, so keep the
                        # 1MB h tiles out of the block-0 weight feed. The
                        # SWDGE launch latency is ~4us, so release tile1's
                        # GENERATION while the ft2/ft3 weight pieces still
                        # stream — its transfer then enters the queue right
                        # behind uc3 instead of 4us late.
                        if i == 1:
                            gp.wait_ge(s_puc[1], 16)
                        elif i == 2:
                            gp.wait_ge(s_wd0, 32)
                    if p >= 1:
                        # WAR on h_sb cols [off, off+wdt): last readers are
                        # pass p-1's gus of the overlapping tiles
                        m = max(_overlaps(p - 1, off, off + wdt), default=-1)
                        gp.wait_ge(s_u, 4 * (ctg_base[p - 1] + m + 1))
                    tsl = slice(pass_tok0[p] + off, pass_tok0[p] + off + wdt)
                    gp.dma_start(
                        h_sb[:, :, :, off : off + wdt], hT_v[:, :, :, tsl]
                    ).then_inc(s_ht[i], 16)
                    off += wdt

            def store_y_tiles(p):
                # stream each finished ct tile out as soon as its yupds
                # land; the last pass goes out in ht-pair pieces
                for j in range(NCT[p]):
                    ctg = ctg_base[p] + j
                    coff = tile_offs[p][j]
                    ctw = tiles[p][j]
                    tsl = slice(pass_tok0[p] + coff, pass_tok0[p] + coff + ctw)
                    if p == NP - 1:
                        for hp in range(4):
                            # hts [2hp, 2hp+2) are covered by yupd op
                            # YUPD_FREE[2hp+1] of this ctg
                            gp.wait_ge(s_yupd, YUPD_N * ctg + YUPD_FREE[2 * hp + 1])
                            gp.dma_start(
                                yT_v[:, 2 * hp : 2 * hp + 2, tsl],
                                y_sb[:, 2 * hp : 2 * hp + 2, coff : coff + ctw],
                            ).then_inc(s_yd[j], 16)
                    else:
                        gp.wait_ge(s_yupd, YUPD_N * (ctg + 1))
                        gp.dma_start(
                            yT_v[:, :, tsl], y_sb[:, :, coff : coff + ctw]
                        ).then_inc(s_yd[j], 16)

            load_h(0)
            if NP > 1:
                load_h(1)
            for p in range(2, NP):
                store_y_tiles(p - 2)
                load_h(p)
            for p in range(max(NP - 2, 0), NP - 1):
                store_y_tiles(p)
            # the FINAL pass's stores are issued from the sync queue:
            # HWDGE generation (0.62us) beats SWDGE (1.1us) on the
            # end-of-run critical path, and sync is idle by then

        # ---------------- PE stream (one ct-tile lookahead) ----------------
        @block.tensor
        def _(te):
            def gu(ctg):
                p, ct, ctw, coff = ctg_pfc[ctg]
                bs = pass_wb[p]
                buf = bs % 2
                if p == 0 and ct == 0:
                    te.wait_ge(s_h0, 16)
                elif ct == 0:
                    te.wait_ge(s_ht[0], 16 * (ht_cnt[p][0] - 1))
                else:
                    te.wait_ge(s_ht[ct], 16 * ht_cnt[p][ct])
                if ct == 0 and bs > 0 and p == blk_pass_first[bs]:
                    te.wait_ge(s_w1 if bs % 2 == 1 else s_w0, swp_need(bs))
                use_pre = p >= 1 and ct == 0
                csl = slice(coff, coff + ctw)

                def rhs_main(kp):
                    if use_pre:
                        return h_pre[:, 2 * kp : 2 * kp + 2, 1, :ctw]
                    return h_sb[:, 2 * kp : 2 * kp + 2, 1, csl]

                def rhs_cross(k):
                    if use_pre:
                        return h_pre[:, k, :, :ctw]
                    return h_sb[:, k, :, csl]

                blk0 = p == 0 and ct == 0
                for ft in range(FT_PER):
                    gi = ctg * 4 + ft
                    gb = gi % 2
                    if blk0:
                        # startup: mains need only the hi h plane + the gm
                        # piece; the cross wait sits MID-GROUP so the PE
                        # starts ~3us before the gc piece lands
                        te.wait_ge(s_pg[ft], 16)
                    if gi >= 2:
                        te.wait_ge(s_silu, gi - 1)
                    for kp in range(KP):
                        nc.tensor.matmul(
                            g_ps[gb][:, :ctw],
                            wgm_sb[:, buf, ft, kp],
                            rhs_main(kp),
                            start=(kp == 0),
                            stop=False,
                            perf_mode=DR,
                        )
                    if blk0:
                        warm(WARM_PAD)
                        if ft == 0:
                            te.wait_ge(s_h0b, 16)
                        te.wait_ge(s_pgc[ft], 16)
                    for k in range(KT):
                        mm = nc.tensor.matmul(
                            g_ps[gb][:, :ctw],
                            wgc_sb[:, buf, ft, k],
                            rhs_cross(k),
                            start=False,
                            stop=(k == KT - 1),
                            perf_mode=DR,
                        )
                        if k == KT - 1:
                            mm.then_inc(s_g, 1)
                    if blk0:
                        warm(WARM_PAD)
                        te.wait_ge(s_pu[ft], 16)
                    if gi >= 2:
                        te.wait_ge(s_mul, gi - 1)
                    for kp in range(KP):
                        nc.tensor.matmul(
                            u_ps[gb][:, :ctw],
                            wum_sb[:, buf, ft, kp],
                            rhs_main(kp),
                            start=(kp == 0),
                            stop=False,
                            perf_mode=DR,
                        )
                    if blk0:
                        te.wait_ge(s_puc[ft], 16)
                    for k in range(KT):
                        mm = nc.tensor.matmul(
                            u_ps[gb][:, :ctw],
                            wuc_sb[:, buf, ft, k],
                            rhs_cross(k),
                            start=False,
                            stop=(k == KT - 1),
                            perf_mode=DR,
                        )
                        if k == KT - 1:
                            mm.then_inc(s_u, 1)

            def down(ctg):
                p, ct, ctw, coff = ctg_pfc[ctg]
                bs = pass_wb[p]
                buf = bs % 2
                ab = ctg % 2
                if ct == 0 and p == blk_pass_first[bs]:
                    if bs == 0:
                        te.wait_ge(s_wd0, 32)
                    else:
                        te.wait_ge(s_w1 if bs % 2 == 1 else s_w0, swp_need(bs))
                te.wait_ge(s_sub, 4 * (ctg + 1))
                te.wait_ge(s_a1, 4 * (ctg + 1))
                for ht in range(HT):
                    di = ctg * 8 + ht
                    db = di % 4
                    if di >= 4:
                        # bank db last written by down di-4; s_yupd counts
                        # the 5 copy-ops per ctg (see YUPD_FREE)
                        cp, hp = (di - 4) // 8, (di - 4) % 8
                        te.wait_ge(s_yupd, YUPD_N * cp + YUPD_FREE[hp])
                    for fp in range(FPAIR):
                        nc.tensor.matmul(
                            yp_ps[:, db, :ctw],
                            wdm_sb[:, buf, ht, fp],
                            act8[:, ab, 2 * fp : 2 * fp + 2, 0, :ctw],
                            start=(fp == 0),
                            stop=False,
                            perf_mode=DR,
                        )
                    for ft in range(FT_PER):
                        mm = nc.tensor.matmul(
                            yp_ps[:, db, :ctw],
                            wdc_sb[:, buf, ht, ft],
                            act8[:, ab, ft, 1:3, :ctw],
                            start=False,
                            stop=(ft == FT_PER - 1),
                            perf_mode=DR,
                        )
                        if ft == FT_PER - 1:
                            mm.then_inc(s_down, 1)

            def warm(n):
                # p-state keep-alive: zero matmuls into yp bank 0 (reset by
                # down(0)'s start=True later). Any PE idle gap drops the
                # clock to 1.2GHz for 3us; these span the startup DMA waits
                # so the real stream starts and stays at 2.4GHz.
                for _ in range(n):
                    nc.tensor.matmul(
                        yp_ps[:, 0, :512],
                        warm_sb[:, :, :128],
                        warm_sb[:, :, :],
                        start=True,
                        stop=True,
                        perf_mode=DR,
                    )

            te.wait_ge(s_warm, 1)
            warm(WARM_N1)
            gu(0)
            for ctg in range(TOTAL_CT):
                if ctg + 1 < TOTAL_CT:
                    gu(ctg + 1)
                down(ctg)

        # ---------------- ACT stream (silu + fp8 casts) ------------------
        @block.scalar
        def _(sc):
            def silu_one(ctg, ft):
                ab = ctg % 2
                ctw = ctg_pfc[ctg][2]
                gi = ctg * 4 + ft
                gb = gi % 2
                if ft == 0 and ctg >= 2:
                    # WAR on s_sb[ab]: DVE subs of ctg-2 done
                    sc.wait_ge(s_sub, 4 * (ctg - 1))
                sc.wait_ge(s_g, gi + 1)
                nc.scalar.activation(
                    s_sb[:, ab, ft, :ctw],
                    g_ps[gb][:, :ctw],
                    ACT_FUNC,
                    scale=1.0 / SC,
                ).then_inc(s_silu, 1)

            def casts_one(ctg, ft):
                ab = ctg % 2
                ctw = ctg_pfc[ctg][2]
                gi = ctg * 4 + ft
                if ft == 0 and ctg >= 2:
                    # WAR on act8[ab]: down mms of ctg-2 done
                    sc.wait_ge(s_down, 8 * (ctg - 1))
                sc.wait_ge(s_mul, gi + 1)
                nc.scalar.activation(
                    act8[:, ab, ft, 0, :ctw],
                    s_sb[:, ab, ft, :ctw],
                    mybir.ActivationFunctionType.Copy,
                ).then_inc(s_c1, 1)
                nc.scalar.activation(
                    act8[:, ab, ft, 2, :ctw],
                    s_sb[:, ab, ft, :ctw],
                    mybir.ActivationFunctionType.Copy,
                    scale=0.125,
                ).then_inc(s_a1, 1)

            # h ct0 from the ACT engine's HWDGE queue: runs in parallel
            # with the sync queue's block-0 weight pieces (startup is
            # feed-limited; ACT's first silu is ~4us in anyway). Split by
            # plane: the HI plane (v=1) is all the main-term DRs need, so
            # the first matmul isn't gated on the full 1MB tile.
            w0 = tiles[0][0]
            sc.dma_start(h_sb[:, :, 1, :w0], hT_v[:, :, 1, :w0]).then_inc(s_h0, 16)
            sc.dma_start(h_sb[:, :, 0, :w0], hT_v[:, :, 0, :w0]).then_inc(s_h0b, 16)

            def yupd_act(ctg):
                # tail-only: ctg T-2's y copies on ACT (DVE is congested
                # with muls/subs there and the back-to-back final downs
                # would stall ~4us on the psum-bank WAR)
                p, ct, ctw, coff = ctg_pfc[ctg]
                csl = slice(coff, coff + ctw)
                # op 1 (ht0) runs on the DVE (it has tail slack); ACT does
                # ops 2..6 so its critical pair ops land ~0.7us earlier
                for j, (h0, h1) in enumerate(YUPD_HT):
                    if j == 0:
                        continue
                    sc.wait_ge(s_down, ctg * 8 + h1)
                    if j == 1:
                        # cross-engine s_yupd handover: prove DVE's op 1
                        # (and everything before) has landed
                        sc.wait_ge(s_yupd, YUPD_N * ctg + 1)
                    nc.scalar.activation(
                        y_sb[:, h0:h1, csl],
                        yp_ps[:, h0 % 4 : (h1 - 1) % 4 + 1, :ctw],
                        mybir.ActivationFunctionType.Copy,
                        scale=1.0 / SC,
                    ).then_inc(s_yupd, 1)

            for ctg in range(TOTAL_CT):
                # interleave: casts lag silus by one ft so ACT never waits
                # long on the DVE muls
                silu_one(ctg, 0)
                silu_one(ctg, 1)
                casts_one(ctg, 0)
                silu_one(ctg, 2)
                casts_one(ctg, 1)
                silu_one(ctg, 3)
                casts_one(ctg, 2)
                casts_one(ctg, 3)
            if TOTAL_CT >= 2:
                yupd_act(TOTAL_CT - 2)

        # ---------------- DVE stream (mul + sub + y copy) ------------------
        @block.vector
        def _(ve):
            def mul_one(ctg, ft):
                ab = ctg % 2
                ctw = ctg_pfc[ctg][2]
                gi = ctg * 4 + ft
                gb = gi % 2
                ve.wait_ge(s_silu, gi + 1)
                ve.wait_ge(s_u, gi + 1)
                nc.vector.tensor_mul(
                    s_sb[:, ab, ft, :ctw],
                    s_sb[:, ab, ft, :ctw],
                    u_ps[gb][:, :ctw],
                ).then_inc(s_mul, 1)

            def sub_one(ctg, ft):
                ab = ctg % 2
                ctw = ctg_pfc[ctg][2]
                gi = ctg * 4 + ft
                ve.wait_ge(s_c1, gi + 1)
                nc.vector.tensor_sub(
                    act8[:, ab, ft, 1, :ctw],
                    s_sb[:, ab, ft, :ctw],
                    act8[:, ab, ft, 0, :ctw],
                ).then_inc(s_sub, 1)

            def muls(ctg):
                for ft in range(FT_PER):
                    mul_one(ctg, ft)

            def subs(ctg):
                for ft in range(FT_PER):
                    sub_one(ctg, ft)

            def yupd(ctg, only_first=False):
                p, ct, ctw, coff = ctg_pfc[ctg]
                csl = slice(coff, coff + ctw)
                for j, (h0, h1) in enumerate(YUPD_HT):
                    if only_first and j > 0:
                        break
                    ve.wait_ge(s_down, ctg * 8 + h1)
                    if j == 0 and TOTAL_CT >= 2 and ctg == TOTAL_CT - 1:
                        # cross-engine s_yupd handover: prove ACT's T-2
                        # increments have landed before DVE adds its own
                        ve.wait_ge(s_yupd, YUPD_N * ctg)
                    if j == 0 and p > 0:
                        # WAR on y_sb cols: stores of ALL overlapping tiles
                        # through pass p-1 must have drained
                        ov = _overlaps(p - 1, coff, coff + ctw) or range(SLOTS)
                        for i in ov:
                            ve.wait_ge(s_yd[i], 16 * yd_cnt[p - 1][i])
                    nc.vector.tensor_scalar_mul(
                        y_sb[:, h0:h1, csl],
                        yp_ps[:, h0 % 4 : (h1 - 1) % 4 + 1, :ctw],
                        1.0 / SC,
                    ).then_inc(s_yupd, 1)

            # Order per steady-state step c: [muls(c+1), yupd(c), subs(c+1)].
            # yupd(c) must start the moment down(c) begins on the PE (its
            # copies gate down(c)'s later hts via the 4-bank yp WAR), so it
            # cannot sit behind subs(c+1) whose casts depend on gu(c+1)'s
            # end. subs(c) are issued in step c-1, before yupd(c-1) would
            # block on down(c-1) -> no engine-order deadlock.
            nc.vector.memset(warm_sb[:, :, :], 0).then_inc(s_warm, 1)
            muls(0)
            subs(0)
            for ctg in range(TOTAL_CT):
                if ctg + 1 < TOTAL_CT:
                    muls(ctg + 1)
                if TOTAL_CT >= 2 and ctg == TOTAL_CT - 2:
                    # T-2: DVE does op 1 (ht0) only — ACT (which also has
                    # the final casts queued) takes ops 2..6, landing its
                    # critical pairs ~0.7us earlier
                    yupd(ctg, only_first=True)
                else:
                    yupd(ctg)
                if ctg + 1 < TOTAL_CT:
                    subs(ctg + 1)

    return nc


# ----------------------------------------------------------------------------
# Host side
# ----------------------------------------------------------------------------


def _q8(x):
    """fp8 e4m3 quantize (round-to-nearest-even), back to f32."""
    return x.astype(E4M3).astype(np.float32)


def _q8r(x):
    """fp8 e4m3 quantize, keep fp8 dtype."""
    return np.ascontiguousarray(x.astype(E4M3))


def _route(h, Wr, topk):
    """Exact fp32 replica of the reference router. Returns sel [T,k], w [T,k]."""
    logits = h @ Wr.T  # [T, E]
    logits = logits.astype(np.float32)
    m = logits.max(axis=-1, keepdims=True)
    e = np.exp(logits - m)
    p = e / e.sum(axis=-1, keepdims=True)
    sel = np.argsort(-p, axis=-1, kind="stable")[:, :topk]  # ties -> lower idx
    w = np.take_along_axis(p, sel, axis=-1)
    if topk != 1:
        w = w / w.sum(axis=-1, keepdims=True)
    return sel, w.astype(np.float32)


def _pack_gu(WT):
    """Gate/up weight packing. WT: [H, FSH] f32 (already pre-scaled).
    Returns (main, cross) fp8 blobs shaped [128, FT*KP*2*128] and
    [128, FT*KT*2*128]."""
    W1 = _q8(WT)
    W1s = W1 * SC  # exact in fp8 (power-of-two, no overflow)
    W2s = _q8(SC * (WT - W1))
    # main: [p, ft, kp, i, fi] = W1s[(2kp+i)*128+p, ft*128+fi]
    main = W1s.reshape(KP, 2, 128, FT_PER, 128).transpose(2, 3, 0, 1, 4)
    # cross: [p, ft, k, v, fi] with v0 = W1 (hi, unscaled), v1 = W2s
    cross = np.stack([W1, W2s]).reshape(2, KT, 128, FT_PER, 128).transpose(
        2, 3, 1, 0, 4
    )
    return (
        _q8r(main).reshape(128, -1),
        _q8r(cross).reshape(128, -1),
    )


def _pack_d(DT):
    """Down weight packing. DT: [FSH, H] f32 (pre-scaled by 4).
    Returns (main, cross) fp8 blobs [128, HT*FPAIR*2*128], [128, HT*FT*2*128]."""
    D1 = _q8(DT)
    D2s = _q8(8.0 * (DT - D1))
    # main: [p, ht, fp, i, hi] = D1[(2fp+i)*128+p, ht*128+hi]
    main = D1.reshape(FPAIR, 2, 128, HT, 128).transpose(2, 3, 0, 1, 4)
    # cross: [p, ht, ft, v, hi] with v0 = D1 (hi), v1 = D2s
    cross = np.stack([D1, D2s]).reshape(2, FT_PER, 128, HT, 128).transpose(
        2, 3, 1, 0, 4
    )
    return (
        _q8r(main).reshape(128, -1),
        _q8r(cross).reshape(128, -1),
    )


def kernel(x, Wr, Wg, Wu, Wd, topk):
    topk = int(topk)
    x = np.asarray(x, dtype=np.float32)
    Wr = np.asarray(Wr, dtype=np.float32)
    Wg = np.asarray(Wg, dtype=np.float32)
    Wu = np.asarray(Wu, dtype=np.float32)
    Wd = np.asarray(Wd, dtype=np.float32)

    T = x.shape[0] * x.shape[1]
    h = np.ascontiguousarray(x.reshape(T, H))

    sel, w = _route(h, Wr, topk)

    idx = [None] * E
    wts = [None] * E
    for e in range(E):
        tok, kk = np.nonzero(sel == e)
        idx[e] = tok
        wts[e] = w[tok, kk]
    counts = [len(i) for i in idx]

    present = [e for e in range(E) if counts[e] > 0]
    sizes = [max(256, ((counts[e] + 31) // 32) * 32) for e in present]
    TC = sum(sizes)
    tok0 = [sum(sizes[:i]) for i in range(len(sizes))]

    nc = build_program(sizes)

    # hT: all experts' tokens grouped and padded — identical on every core.
    # fp8 split: v1 = h1 = fp8(h), v0 = h2s = fp8(32*(h-h1)).
    hTfull = h.T  # [H, T] view
    hT = np.zeros((H, TC), dtype=np.float32)
    for i, e in enumerate(present):
        hT[:, tok0[i] : tok0[i] + counts[e]] = hTfull[:, idx[e]]
    h1 = _q8(hT)
    h2s = SC * (hT - h1)
    hpack = np.empty((KT, 2, 128, TC), dtype=E4M3)
    hpack[:, 0] = h2s.reshape(KT, 128, TC).astype(E4M3)
    hpack[:, 1] = h1.reshape(KT, 128, TC).astype(E4M3)
    hT_in = np.ascontiguousarray(hpack).reshape(KT * 2 * 128, TC)

    # per-core weight slices: core k owns F rows [k*FSH,(k+1)*FSH) of every
    # expert, packed into fp8 main/cross blobs (concatenated in block order)
    NWB = len(present)
    in_maps = []
    for k in range(E):
        fs = slice(k * FSH, (k + 1) * FSH)
        gm = np.empty((128, NWB, FT_PER * KP * 2 * 128), dtype=E4M3)
        gc = np.empty((128, NWB, FT_PER * KT * 2 * 128), dtype=E4M3)
        um = np.empty((128, NWB, FT_PER * KP * 2 * 128), dtype=E4M3)
        uc = np.empty((128, NWB, FT_PER * KT * 2 * 128), dtype=E4M3)
        dm = np.empty((128, NWB, HT * FPAIR * 2 * 128), dtype=E4M3)
        dc = np.empty((128, NWB, HT * FT_PER * 2 * 128), dtype=E4M3)
        for i, e in enumerate(present):
            gm[:, i], gc[:, i] = _pack_gu(np.ascontiguousarray(Wg[e, fs, :].T))
            um[:, i], uc[:, i] = _pack_gu(
                np.ascontiguousarray(Wu[e, fs, :].T) * 0.25
            )
            dm[:, i], dc[:, i] = _pack_d(
                np.ascontiguousarray(Wd[e, :, fs].T) * 4.0
            )
        in_maps.append(
            {
                "hT": hT_in,
                "wgm": gm.reshape(128, -1),
                "wgc": gc.reshape(128, -1),
                "wum": um.reshape(128, -1),
                "wuc": uc.reshape(128, -1),
                "wdm": dm.reshape(128, -1),
                "wdc": dc.reshape(128, -1),
            }
        )

    res = run_bass_kernel_spmd(nc, in_maps, core_ids=list(range(E)))

    # sum the 8 partial projections (bf16 -> f32), then combine
    ysum = res.results[0]["yT"].astype(np.float64)
    for k in range(1, E):
        ysum += res.results[k]["yT"].astype(np.float64)
    out = np.zeros((T, H), dtype=np.float32)
    for i, e in enumerate(present):
        cnt = counts[e]
        ye = ysum[:, tok0[i] : tok0[i] + cnt].T  # [cnt, H] f64
        out[idx[e]] += (wts[e][:, None].astype(np.float64) * ye).astype(np.float32)
    return out.reshape(x.shape)
